# revision 22
# baseline (speedup 1.0000x reference)
"""DSAttention TRN2 Bass kernel.

Reference (per batch b, head h, branch):
    z[l,s] = (q[l]·k[s]) * tau[b]/8 + delta[b,s]/8        (causal: s <= l)
    A = softmax_s(z);  O = A @ V
    out = m*O_edit + (1-m)*O_null,  m = soft_mask[b,l]

Sharding: B*H = 16 (b,h) slices -> 8 cores x 2 heads. Same SPMD program on
every core; core c gets b = c//4, heads 2*(c%4), 2*(c%4)+1.

Per-core algorithm (transposed-score flash attention, software-pipelined):
  - Host pre-packs per head in bf16: qt = [Q^T; Q^T] (dup) and
    kt = [K_e^T; K_n^T] [128, L] so the two branches' QK^T matmuls run
    row-packed (tile_position (0,0)/(64,0)), and V with a ones column
    appended, pre-gathered to [128, NT, 65] for a contiguous DMA.
  - scores^T tile [s:128, l:<=512] per (S-tile, L-chunk), trimmed to the
    causal region; exp on ACT with fused scale=tau/8 and bias=delta_s/8,
    both branches in one ACTIVATE, output bf16; diagonal 128x128 block
    masked post-exp with a triangular 0/1 tile (edit branch on DVE, null
    branch on the otherwise-idle GPSIMD so neither serializes the other).
  - O^T[65, 512] accumulates AV matmuls over S-tiles (ones column => row 64
    of O^T is the softmax denominator).
  - Pipeline: the whole (bh, lc, js) stream is flattened; AV(item) is
    emitted one stream-item late so the PE queue is QK(j) QK(j+1) AV(j) ...
    and ACT's exp(j) overlaps PE work instead of serializing the chain.
    Each L-chunk's epilogue (PSUM->SBUF copy, PE-transpose O^T -> [l,65],
    reciprocal of the denominator, soft_mask blend, DMA out) is split into
    per-128-row actions drained one per subsequent stream item, so the
    transposes never stall the PE queue on the DVE chain.

REPEAT > 1 wraps the whole per-core program in a hardware For_i loop; used
by the timing harness to measure per-iteration HW time from wall-clock
deltas (transfers cancel).
"""

import contextlib

import numpy as np
import ml_dtypes

import concourse.bass as bass
import concourse.tile as tile
from concourse import bacc, mybir
from concourse.bass_utils import run_bass_kernel_spmd

B, L, S, H, E, D = 2, 2048, 2048, 8, 64, 64
NCORES = 8
HPC = 2            # heads per core
NT = 16            # 128-row tiles in 2048
LCH = 4            # 512-wide L chunks
F32 = mybir.dt.float32
BF16 = mybir.dt.bfloat16
NPBF16 = ml_dtypes.bfloat16
EXPF = mybir.ActivationFunctionType.Exp
MUL = mybir.AluOpType.mult
ADD = mybir.AluOpType.add

TRACE = False
LAST_EXEC_NS = None
PTS_BUFS = 6
REPEAT = 1

_NC = None


def _build():
    nc = bacc.Bacc("TRN2")
    qt_p = nc.declare_dram_parameter("qt", [HPC, 128, L], BF16, isOutput=False)
    kt_p = nc.declare_dram_parameter("kt", [HPC, 128, S], BF16, isOutput=False)
    # v/vn pre-gathered on host to [128, NT, D+1] so the DMA is one
    # contiguous 2080B line per partition instead of 16 strided 130B rows.
    v_p = nc.declare_dram_parameter(
        "v", [HPC, 128, NT, D + 1], BF16, isOutput=False
    )
    vn_p = nc.declare_dram_parameter(
        "vn", [HPC, 128, NT, D + 1], BF16, isOutput=False
    )
    # consts packed: [st | cdelta | mt | mt1] = [128, 1+NT+NT+NT]
    cst_p = nc.declare_dram_parameter(
        "csts", [128, 1 + 3 * NT], F32, isOutput=False
    )
    id_p = nc.declare_dram_parameter("ident", [128, 65], BF16, isOutput=False)
    mk_p = nc.declare_dram_parameter("mask", [128, 128], BF16, isOutput=False)
    out_p = nc.declare_dram_parameter("out", [HPC, L, D], F32, isOutput=True)
    params = (qt_p, kt_p, v_p, vn_p, cst_p, id_p, mk_p, out_p)

    with tile.TileContext(nc) as tc:
        with (
            tc.tile_pool(name="const", bufs=1) as const,
            tc.tile_pool(name="big", bufs=1) as big,
            tc.tile_pool(name="pts", bufs=PTS_BUFS) as pts,
            tc.tile_pool(name="osb", bufs=3) as osb,
            tc.tile_pool(name="sml", bufs=8) as sml,
            tc.tile_pool(name="ob", bufs=4) as ob,
            tc.tile_pool(name="ps_pt", bufs=2, space="PSUM") as ps_pt,
            tc.tile_pool(name="ps_oac", bufs=1, space="PSUM") as ps_oac,
            tc.tile_pool(name="ps_tr", bufs=2, space="PSUM") as ps_tr,
        ):
            pools = (const, big, pts, osb, sml, ob, ps_pt, ps_oac, ps_tr)
            rep = (
                tc.For_i(0, REPEAT, 1)
                if REPEAT > 1
                else contextlib.nullcontext()
            )
            with rep:
                _body(nc, pools, params)
    if not nc.is_finalized():
        nc.finalize()
    return nc


def _body(nc, pools, params):
    const, big, pts, osb, sml, ob, ps_pt, ps_oac, ps_tr = pools
    qt_p, kt_p, v_p, vn_p, cst_p, id_p, mk_p, out_p = params

    # DMA issue order is the critical path at kernel start (HWDGE generates
    # one descriptor set per ~625ns, serially). Order by first-use time.
    tiles = {}
    for bh in range(HPC):
        qt = big.tile([128, L], BF16, tag=f"qt{bh}")
        kt = big.tile([128, S], BF16, tag=f"kt{bh}")
        ve = big.tile([128, NT, D + 1], BF16, tag=f"ve{bh}")
        vn = big.tile([128, NT, D + 1], BF16, tag=f"vn{bh}")
        tiles[bh] = (qt, kt, ve, vn)

    qt0, kt0, ve0, vn0 = tiles[0]
    qt1, kt1, ve1, vn1 = tiles[1]
    csts = const.tile([128, 1 + 3 * NT], F32, tag="csts")
    mask = const.tile([128, 128], BF16, tag="mask")
    ident = const.tile([128, 65], BF16, tag="ident")

    nc.sync.dma_start(out=kt0[:, 0:1024], in_=kt_p[0, :, 0:1024])
    nc.sync.dma_start(out=qt0[:, 0:1024], in_=qt_p[0, :, 0:1024])
    nc.sync.dma_start(out=csts, in_=cst_p[:])
    st = csts[:, 0:1]
    cdelta = csts[:, 1 : 1 + NT]
    mt = csts[:, 1 + NT : 1 + 2 * NT]
    mt1 = csts[:, 1 + 2 * NT : 1 + 3 * NT]
    # Warmup activation: depends only on csts; hoists the implicit exp
    # activation-table load off the first real exp's critical path.
    warm = sml.tile([128, 1], F32, tag="sml")
    nc.scalar.activation(out=warm, in_=st, func=EXPF)
    nc.sync.dma_start(out=mask, in_=mk_p[:])
    nc.sync.dma_start(out=ve0, in_=v_p[0])
    nc.sync.dma_start(out=vn0, in_=vn_p[0])
    nc.sync.dma_start(out=ident, in_=id_p[:])
    nc.sync.dma_start(out=kt0[:, 1024:], in_=kt_p[0, :, 1024:])
    nc.sync.dma_start(out=qt0[:, 1024:], in_=qt_p[0, :, 1024:])
    nc.sync.dma_start(out=kt1, in_=kt_p[1])
    nc.sync.dma_start(out=qt1, in_=qt_p[1])
    nc.sync.dma_start(out=ve1, in_=v_p[1])
    nc.sync.dma_start(out=vn1, in_=vn_p[1])

    # Flattened stream of score tiles, processed in a software pipeline:
    # emit QK+exp for item i, then AV for item i-1, then at most one
    # deferred epilogue action.
    stream = [
        (bh, lc, js)
        for bh in range(HPC)
        for lc in range(LCH)
        for js in range(4 * lc + 4)
    ]

    state = {}   # (bh, lc) -> (oac_e, oac_n)
    sbuf = {}    # stream item -> (pt_sb, off)
    actions = []  # deferred epilogue closures, drained one per item

    def emit_qk_exp(item):
        bh, lc, js = item
        qt, kt, ve, vn = tiles[bh]
        lcb = 512 * lc
        off = max(0, 128 * js - lcb)
        sb = 128 * js
        lsl = slice(lcb + off, lcb + 512)
        pt_ps = ps_pt.tile([128, 2, 512], F32, tag="pt")
        nc.tensor.matmul(
            pt_ps[:, 0, off:512],
            kt[0:64, sb : sb + 128],
            qt[0:64, lsl],
            start=True, stop=True, tile_position=(0, 0),
        )
        nc.tensor.matmul(
            pt_ps[:, 1, off:512],
            kt[64:128, sb : sb + 128],
            qt[64:128, lsl],
            start=True, stop=True, tile_position=(64, 0),
        )
        pt_sb = pts.tile([128, 2, 512], BF16, tag="ptsb")
        nc.scalar.activation(
            out=pt_sb[:, :, off:512],
            in_=pt_ps[:, :, off:512],
            func=EXPF,
            bias=cdelta[:, js : js + 1],
            scale=st,
        )
        if sb >= lcb:  # diagonal tile: mask l < s (one branch per engine)
            nc.vector.tensor_mul(
                pt_sb[:, 0, off : off + 128],
                pt_sb[:, 0, off : off + 128],
                mask,
            )
            nc.gpsimd.tensor_mul(
                pt_sb[:, 1, off : off + 128],
                pt_sb[:, 1, off : off + 128],
                mask,
            )
        sbuf[item] = (pt_sb, off)

    def emit_av(item):
        bh, lc, js = item
        qt, kt, ve, vn = tiles[bh]
        pt_sb, off = sbuf.pop(item)
        if js == 0:
            oac_e = ps_oac.tile([D + 1, 512], F32, tag="oe")
            oac_n = ps_oac.tile([D + 1, 512], F32, tag="on")
            state[(bh, lc)] = (oac_e, oac_n)
        oac_e, oac_n = state[(bh, lc)]
        njs = 4 * lc + 4
        last = js == njs - 1
        nc.tensor.matmul(
            oac_e[:, off:512],
            ve[:, js, :],
            pt_sb[:, 0, off:512],
            start=(js == 0), stop=last,
        )
        nc.tensor.matmul(
            oac_n[:, off:512],
            vn[:, js, :],
            pt_sb[:, 1, off:512],
            start=(js == 0), stop=last,
        )
        if last:
            oe_sb = osb.tile([D + 1, 512], BF16, tag="oesb")
            on_sb = osb.tile([D + 1, 512], BF16, tag="onsb")
            final = (bh, lc) == (HPC - 1, LCH - 1)
            if final:
                # stream ends here: chunk the copies so each epilogue
                # action starts as soon as its 128-col slice is staged
                for t4 in range(4):
                    csl = slice(128 * t4, 128 * t4 + 128)
                    nc.vector.tensor_copy(out=oe_sb[:, csl], in_=oac_e[:, csl])
                    nc.vector.tensor_copy(out=on_sb[:, csl], in_=oac_n[:, csl])
            else:
                # one copy per branch: the next L-chunk's first AV reuses
                # these PSUM banks (oac bufs=1 WAR), keep the release fast.
                # copy_e first: the next chunk's AV_e is the first waiter.
                nc.vector.tensor_copy(out=oe_sb, in_=oac_e)
                nc.vector.tensor_copy(out=on_sb, in_=oac_n)
            for t4 in range(4):
                actions.append(
                    lambda bh=bh, lc=lc, t4=t4, oe_sb=oe_sb, on_sb=on_sb,
                    final=final:
                    emit_epilogue(bh, lc, t4, oe_sb, on_sb, final)
                )

    obufs = {}  # (bh, lc) -> staging tile, flushed in one DMA per L-chunk

    def emit_epilogue(bh, lc, t4, oe_sb, on_sb, final=False):
        lt = 4 * lc + t4
        csl = slice(128 * t4, 128 * t4 + 128)
        tr_e = ps_tr.tile([128, 65], BF16, tag="tr")
        nc.tensor.transpose(tr_e, oe_sb[:, csl], ident[0:65, 0:65])
        tr_n = ps_tr.tile([128, 65], BF16, tag="tr")
        nc.tensor.transpose(tr_n, on_sb[:, csl], ident[0:65, 0:65])
        rec_e = sml.tile([128, 1], F32, tag="sml")
        nc.vector.reciprocal(rec_e, tr_e[:, 64:65])
        rec_n = sml.tile([128, 1], F32, tag="sml")
        nc.vector.reciprocal(rec_n, tr_n[:, 64:65])
        se = sml.tile([128, 1], F32, tag="sml")
        nc.vector.tensor_mul(se, rec_e, mt[:, lt : lt + 1])
        sn = sml.tile([128, 1], F32, tag="sml")
        nc.vector.tensor_mul(sn, rec_n, mt1[:, lt : lt + 1])
        if final:
            # drain phase: per-action DMA so HWDGE generation overlaps the
            # remaining actions' DVE chains instead of serializing after them
            obuf = ob.tile([128, D], F32, tag="obf", name="obuf")
            nc.vector.tensor_scalar_mul(obuf, tr_e[:, 0:64], se)
            nc.vector.scalar_tensor_tensor(
                out=obuf, in0=tr_n[:, 0:64], scalar=sn, in1=obuf,
                op0=MUL, op1=ADD,
            )
            nc.sync.dma_start(
                out=out_p[bh, 128 * lt : 128 * lt + 128, :], in_=obuf
            )
            return
        if t4 == 0:
            obufs[(bh, lc)] = ob.tile([128, 4, D], F32, tag="ob", name="obuf")
        obuf = obufs[(bh, lc)]
        nc.vector.tensor_scalar_mul(obuf[:, t4, :], tr_e[:, 0:64], se)
        nc.vector.scalar_tensor_tensor(
            out=obuf[:, t4, :], in0=tr_n[:, 0:64], scalar=sn, in1=obuf[:, t4, :],
            op0=MUL, op1=ADD,
        )
        if t4 == 3:
            nc.sync.dma_start(
                out=out_p[bh, 512 * lc : 512 * lc + 512, :].rearrange(
                    "(t p) d -> p t d", p=128
                ),
                in_=obufs.pop((bh, lc)),
            )

    prev = None
    for idx, item in enumerate(stream):
        emit_qk_exp(item)
        if prev is not None:
            emit_av(prev)
            # drain at most one epilogue action every other item: an
            # action adds ~110ns of PE transposes, which would eat the
            # whole PE-vs-ACT slack if popped every item
            if actions and idx % 2 == 0:
                actions.pop(0)()
        prev = item
    emit_av(prev)
    while actions:
        actions.pop(0)()


def _host_in_maps(queries, keys, values, keys_null, values_null, tau, delta,
                  soft_mask):
    ident = np.eye(128, dtype=np.float32)[:, 0:65].astype(NPBF16)
    mask = np.triu(np.ones((128, 128))).astype(NPBF16)

    in_maps = []
    for c in range(NCORES):
        b, h0 = c // 4, HPC * (c % 4)
        qt = np.empty((HPC, 128, L), NPBF16)
        kt = np.empty((HPC, 128, S), NPBF16)
        v = np.empty((HPC, 128, NT, D + 1), NPBF16)
        vn = np.empty((HPC, 128, NT, D + 1), NPBF16)
        for bh in range(HPC):
            h = h0 + bh
            qT = queries[b, :, h, :].T.astype(NPBF16)  # [E, L]
            qt[bh, 0:64] = qT
            qt[bh, 64:128] = qT
            kt[bh, 0:64] = keys[b, :, h, :].T.astype(NPBF16)
            kt[bh, 64:128] = keys_null[b, :, h, :].T.astype(NPBF16)
            # [S, D] -> [128 partitions, NT tiles, D(+ones)]
            v[bh, :, :, 0:D] = (
                values[b, :, h, :].reshape(NT, 128, D).transpose(1, 0, 2)
            ).astype(NPBF16)
            v[bh, :, :, D] = 1.0
            vn[bh, :, :, 0:D] = (
                values_null[b, :, h, :].reshape(NT, 128, D).transpose(1, 0, 2)
            ).astype(NPBF16)
            vn[bh, :, :, D] = 1.0
        m_t = np.ascontiguousarray(soft_mask[b].reshape(NT, 128).T)
        csts = np.empty((128, 1 + 3 * NT), np.float32)
        csts[:, 0] = tau[b, 0] / 8.0
        csts[:, 1 : 1 + NT] = (delta[b] / 8.0).reshape(NT, 128).T
        csts[:, 1 + NT : 1 + 2 * NT] = m_t
        csts[:, 1 + 2 * NT :] = 1.0 - m_t
        in_maps.append(
            dict(
                qt=qt, kt=kt, v=v, vn=vn,
                csts=csts,
                ident=ident,
                mask=mask,
            )
        )
    return in_maps


def kernel(queries, keys, values, keys_null, values_null, tau, delta, soft_mask):
    global _NC, LAST_EXEC_NS
    queries = np.asarray(queries, dtype=np.float32)
    keys = np.asarray(keys, dtype=np.float32)
    values = np.asarray(values, dtype=np.float32)
    keys_null = np.asarray(keys_null, dtype=np.float32)
    values_null = np.asarray(values_null, dtype=np.float32)
    tau = np.asarray(tau, dtype=np.float32)
    delta = np.asarray(delta, dtype=np.float32)
    soft_mask = np.asarray(soft_mask, dtype=np.float32)

    if _NC is None:
        _NC = _build()

    in_maps = _host_in_maps(
        queries, keys, values, keys_null, values_null, tau, delta, soft_mask
    )
    res = run_bass_kernel_spmd(
        _NC, in_maps, core_ids=list(range(NCORES)), trace=TRACE
    )
    LAST_EXEC_NS = res.exec_time_ns

    out = np.empty((B, L, H, D), np.float32)
    for c in range(NCORES):
        b, h0 = c // 4, HPC * (c % 4)
        out[b, :, h0 : h0 + HPC, :] = res.results[c]["out"].transpose(1, 0, 2)
    return out


# revision 28
# speedup vs baseline: 1.2107x; 1.2107x over previous
"""DSAttention TRN2 Bass kernel.

Reference (per batch b, head h, branch):
    z[l,s] = (q[l]·k[s]) * tau[b]/8 + delta[b,s]/8        (causal: s <= l)
    A = softmax_s(z);  O = A @ V
    out = m*O_edit + (1-m)*O_null,  m = soft_mask[b,l]

Sharding: B*H = 16 (b,h) slices -> 8 cores x 2 heads. Same SPMD program on
every core; core c gets b = c//4, heads 2*(c%4), 2*(c%4)+1.

Per-core algorithm (transposed-score flash attention, software-pipelined):
  - Host pre-packs per head in bf16: qt = [Q^T; Q^T] (dup) and
    kt = [K_e^T; K_n^T] [128, L] so the two branches' QK^T matmuls run
    row-packed (tile_position (0,0)/(64,0)), and V with a ones column
    appended, pre-gathered to [128, NT, 65] for a contiguous DMA.
  - scores^T tile [s:128, l:<=512] per (S-tile, L-chunk), trimmed to the
    causal region; exp on ACT with fused scale=tau/8 and bias=delta_s/8,
    both branches in one ACTIVATE, output bf16; diagonal 128x128 block
    masked post-exp with a triangular 0/1 tile (edit branch on DVE, null
    branch on the otherwise-idle GPSIMD so neither serializes the other).
  - O^T[65, 512] accumulates AV matmuls over S-tiles (ones column => row 64
    of O^T is the softmax denominator).
  - Pipeline: the whole (bh, lc, js) stream is flattened; AV(item) is
    emitted one stream-item late so the PE queue is QK(j) QK(j+1) AV(j) ...
    and ACT's exp(j) overlaps PE work instead of serializing the chain.
    Each L-chunk's epilogue (PSUM->SBUF copy, PE-transpose O^T -> [l,65],
    reciprocal of the denominator, soft_mask blend, DMA out) is split into
    per-128-row actions drained one per subsequent stream item, so the
    transposes never stall the PE queue on the DVE chain.

REPEAT > 1 wraps the whole per-core program in a hardware For_i loop; used
by the timing harness to measure per-iteration HW time from wall-clock
deltas (transfers cancel).
"""

import contextlib

import numpy as np
import ml_dtypes

import concourse.bass as bass
import concourse.tile as tile
from concourse import bacc, mybir
from concourse.bass_utils import run_bass_kernel_spmd

B, L, S, H, E, D = 2, 2048, 2048, 8, 64, 64
NCORES = 8
HPC = 2            # heads per core
NT = 16            # 128-row tiles in 2048
LCH = 4            # 512-wide L chunks
F32 = mybir.dt.float32
BF16 = mybir.dt.bfloat16
NPBF16 = ml_dtypes.bfloat16
EXPF = mybir.ActivationFunctionType.Exp
MUL = mybir.AluOpType.mult
ADD = mybir.AluOpType.add

TRACE = False
LAST_EXEC_NS = None
PTS_BUFS = 6
REPEAT = 1
# Per-item filler matmul width. The PE p-state ramp (0.65/1.2 GHz until
# 3us of continuous execution, then 2.4 GHz) means a PE that goes idle
# between items runs its matmuls at half speed; a small filler matmul per
# item keeps the PE saturated so the whole stream runs at full clock.
# Output goes to a scratch region of the score PSUM tile that the real
# QK pair either overwrites (start=True) or the exp never reads.
DUMMY_N = 0
BATCH_OUT = True   # one out-DMA per L-chunk (strided) vs four linear DMAs

_NC = None


def _build():
    nc = bacc.Bacc("TRN2")
    qt_p = nc.declare_dram_parameter("qt", [HPC, 128, L], BF16, isOutput=False)
    kt_p = nc.declare_dram_parameter("kt", [HPC, 128, S], BF16, isOutput=False)
    # v/vn pre-gathered on host to [128, NT, D+1] so the DMA is one
    # contiguous 2080B line per partition instead of 16 strided 130B rows.
    v_p = nc.declare_dram_parameter(
        "v", [HPC, 128, NT, D + 1], BF16, isOutput=False
    )
    vn_p = nc.declare_dram_parameter(
        "vn", [HPC, 128, NT, D + 1], BF16, isOutput=False
    )
    # consts packed: [st | cdelta | mt | mt1] = [128, 1+NT+NT+NT]
    cst_p = nc.declare_dram_parameter(
        "csts", [128, 1 + 3 * NT], F32, isOutput=False
    )
    id_p = nc.declare_dram_parameter("ident", [128, 65], BF16, isOutput=False)
    mk_p = nc.declare_dram_parameter("mask", [128, 128], BF16, isOutput=False)
    out_p = nc.declare_dram_parameter("out", [HPC, L, D], F32, isOutput=True)
    params = (qt_p, kt_p, v_p, vn_p, cst_p, id_p, mk_p, out_p)

    with tile.TileContext(nc) as tc:
        with (
            tc.tile_pool(name="const", bufs=1) as const,
            tc.tile_pool(name="big", bufs=1) as big,
            tc.tile_pool(name="pts", bufs=PTS_BUFS) as pts,
            tc.tile_pool(name="osb", bufs=3) as osb,
            tc.tile_pool(name="sml", bufs=8) as sml,
            tc.tile_pool(name="ob", bufs=4) as ob,
            tc.tile_pool(name="ps_pt", bufs=2, space="PSUM") as ps_pt,
            tc.tile_pool(name="ps_oac", bufs=1, space="PSUM") as ps_oac,
            tc.tile_pool(name="ps_tr", bufs=2, space="PSUM") as ps_tr,
        ):
            pools = (const, big, pts, osb, sml, ob, ps_pt, ps_oac, ps_tr)
            rep = (
                tc.For_i(0, REPEAT, 1)
                if REPEAT > 1
                else contextlib.nullcontext()
            )
            with rep:
                _body(nc, pools, params)
    if not nc.is_finalized():
        nc.finalize()
    return nc


def _body(nc, pools, params):
    const, big, pts, osb, sml, ob, ps_pt, ps_oac, ps_tr = pools
    qt_p, kt_p, v_p, vn_p, cst_p, id_p, mk_p, out_p = params

    # DMA issue order is the critical path at kernel start (HWDGE generates
    # one descriptor set per ~625ns, serially). Order by first-use time.
    tiles = {}
    for bh in range(HPC):
        qt = big.tile([128, L], BF16, tag=f"qt{bh}")
        kt = big.tile([128, S], BF16, tag=f"kt{bh}")
        ve = big.tile([128, NT, D + 1], BF16, tag=f"ve{bh}")
        vn = big.tile([128, NT, D + 1], BF16, tag=f"vn{bh}")
        tiles[bh] = (qt, kt, ve, vn)

    qt0, kt0, ve0, vn0 = tiles[0]
    qt1, kt1, ve1, vn1 = tiles[1]
    csts = const.tile([128, 1 + 3 * NT], F32, tag="csts")
    mask = const.tile([128, 128], BF16, tag="mask")
    ident = const.tile([128, 65], BF16, tag="ident")

    nc.sync.dma_start(out=kt0[:, 0:1024], in_=kt_p[0, :, 0:1024])
    nc.sync.dma_start(out=qt0[:, 0:1024], in_=qt_p[0, :, 0:1024])
    nc.sync.dma_start(out=csts, in_=cst_p[:])
    st = csts[:, 0:1]
    cdelta = csts[:, 1 : 1 + NT]
    mt = csts[:, 1 + NT : 1 + 2 * NT]
    mt1 = csts[:, 1 + 2 * NT : 1 + 3 * NT]
    # Warmup activation: depends only on csts; hoists the implicit exp
    # activation-table load off the first real exp's critical path.
    warm = sml.tile([128, 1], F32, tag="sml")
    nc.scalar.activation(out=warm, in_=st, func=EXPF)
    nc.sync.dma_start(out=mask, in_=mk_p[:])
    nc.sync.dma_start(out=ve0, in_=v_p[0])
    nc.sync.dma_start(out=vn0, in_=vn_p[0])
    nc.sync.dma_start(out=ident, in_=id_p[:])
    nc.sync.dma_start(out=kt0[:, 1024:], in_=kt_p[0, :, 1024:])
    nc.sync.dma_start(out=qt0[:, 1024:], in_=qt_p[0, :, 1024:])
    nc.sync.dma_start(out=kt1, in_=kt_p[1])
    nc.sync.dma_start(out=qt1, in_=qt_p[1])
    nc.sync.dma_start(out=ve1, in_=v_p[1])
    nc.sync.dma_start(out=vn1, in_=vn_p[1])

    # Flattened stream of score tiles, processed in a software pipeline:
    # emit QK+exp for item i, then AV for item i-1, then at most one
    # deferred epilogue action.
    stream = [
        (bh, lc, js)
        for bh in range(HPC)
        for lc in range(LCH)
        for js in range(4 * lc + 4)
    ]

    state = {}   # (bh, lc) -> (oac_e, oac_n)
    sbuf = {}    # stream item -> (pt_sb, off)
    actions = []  # deferred epilogue closures, drained one per item

    def emit_qk_exp(item):
        bh, lc, js = item
        qt, kt, ve, vn = tiles[bh]
        lcb = 512 * lc
        off = max(0, 128 * js - lcb)
        sb = 128 * js
        lsl = slice(lcb + off, lcb + 512)
        pt_ps = ps_pt.tile([128, 2, 512], F32, tag="pt")
        if DUMMY_N:
            nc.tensor.matmul(
                pt_ps[:, 0, 0:DUMMY_N],
                kt[0:64, 0:128],
                qt[0:64, 0:DUMMY_N],
                start=True, stop=True, tile_position=(0, 0),
            )
        nc.tensor.matmul(
            pt_ps[:, 0, off:512],
            kt[0:64, sb : sb + 128],
            qt[0:64, lsl],
            start=True, stop=True, tile_position=(0, 0),
        )
        nc.tensor.matmul(
            pt_ps[:, 1, off:512],
            kt[64:128, sb : sb + 128],
            qt[64:128, lsl],
            start=True, stop=True, tile_position=(64, 0),
        )
        pt_sb = pts.tile([128, 2, 512], BF16, tag="ptsb")
        nc.scalar.activation(
            out=pt_sb[:, :, off:512],
            in_=pt_ps[:, :, off:512],
            func=EXPF,
            bias=cdelta[:, js : js + 1],
            scale=st,
        )
        if sb >= lcb:  # diagonal tile: mask l < s (one branch per engine)
            nc.vector.tensor_mul(
                pt_sb[:, 0, off : off + 128],
                pt_sb[:, 0, off : off + 128],
                mask,
            )
            nc.gpsimd.tensor_mul(
                pt_sb[:, 1, off : off + 128],
                pt_sb[:, 1, off : off + 128],
                mask,
            )
        sbuf[item] = (pt_sb, off)

    def emit_av(item):
        bh, lc, js = item
        qt, kt, ve, vn = tiles[bh]
        pt_sb, off = sbuf.pop(item)
        if js == 0:
            oac_e = ps_oac.tile([D + 1, 512], F32, tag="oe")
            oac_n = ps_oac.tile([D + 1, 512], F32, tag="on")
            state[(bh, lc)] = (oac_e, oac_n)
        oac_e, oac_n = state[(bh, lc)]
        njs = 4 * lc + 4
        last = js == njs - 1
        nc.tensor.matmul(
            oac_e[:, off:512],
            ve[:, js, :],
            pt_sb[:, 0, off:512],
            start=(js == 0), stop=last,
        )
        nc.tensor.matmul(
            oac_n[:, off:512],
            vn[:, js, :],
            pt_sb[:, 1, off:512],
            start=(js == 0), stop=last,
        )
        if last:
            oe_sb = osb.tile([D + 1, 512], BF16, tag="oesb")
            on_sb = osb.tile([D + 1, 512], BF16, tag="onsb")
            final = (bh, lc) == (HPC - 1, LCH - 1)
            if final:
                # stream ends here: chunk the copies so each epilogue
                # action starts as soon as its 128-col slice is staged
                for t4 in range(4):
                    csl = slice(128 * t4, 128 * t4 + 128)
                    nc.vector.tensor_copy(out=oe_sb[:, csl], in_=oac_e[:, csl])
                    nc.vector.tensor_copy(out=on_sb[:, csl], in_=oac_n[:, csl])
            else:
                # one copy per branch: the next L-chunk's first AV reuses
                # these PSUM banks (oac bufs=1 WAR), keep the release fast.
                # copy_e first: the next chunk's AV_e is the first waiter.
                nc.vector.tensor_copy(out=oe_sb, in_=oac_e)
                nc.vector.tensor_copy(out=on_sb, in_=oac_n)
            for t4 in range(4):
                actions.append(
                    lambda bh=bh, lc=lc, t4=t4, oe_sb=oe_sb, on_sb=on_sb,
                    final=final:
                    emit_epilogue(bh, lc, t4, oe_sb, on_sb, final)
                )

    obufs = {}  # (bh, lc) -> staging tile, flushed in one DMA per L-chunk

    def emit_epilogue(bh, lc, t4, oe_sb, on_sb, final=False):
        lt = 4 * lc + t4
        csl = slice(128 * t4, 128 * t4 + 128)
        tr_e = ps_tr.tile([128, 65], BF16, tag="tr")
        nc.tensor.transpose(tr_e, oe_sb[:, csl], ident[0:65, 0:65])
        tr_n = ps_tr.tile([128, 65], BF16, tag="tr")
        nc.tensor.transpose(tr_n, on_sb[:, csl], ident[0:65, 0:65])
        rec_e = sml.tile([128, 1], F32, tag="sml")
        nc.vector.reciprocal(rec_e, tr_e[:, 64:65])
        rec_n = sml.tile([128, 1], F32, tag="sml")
        nc.vector.reciprocal(rec_n, tr_n[:, 64:65])
        se = sml.tile([128, 1], F32, tag="sml")
        nc.vector.tensor_mul(se, rec_e, mt[:, lt : lt + 1])
        sn = sml.tile([128, 1], F32, tag="sml")
        nc.vector.tensor_mul(sn, rec_n, mt1[:, lt : lt + 1])
        if final or not BATCH_OUT:
            # drain phase: per-action DMA so HWDGE generation overlaps the
            # remaining actions' DVE chains instead of serializing after them
            obuf = ob.tile([128, D], F32, tag="obf", name="obuf")
            nc.vector.tensor_scalar_mul(obuf, tr_e[:, 0:64], se)
            nc.vector.scalar_tensor_tensor(
                out=obuf, in0=tr_n[:, 0:64], scalar=sn, in1=obuf,
                op0=MUL, op1=ADD,
            )
            nc.sync.dma_start(
                out=out_p[bh, 128 * lt : 128 * lt + 128, :], in_=obuf
            )
            return
        if t4 == 0:
            obufs[(bh, lc)] = ob.tile([128, 4, D], F32, tag="ob", name="obuf")
        obuf = obufs[(bh, lc)]
        nc.vector.tensor_scalar_mul(obuf[:, t4, :], tr_e[:, 0:64], se)
        nc.vector.scalar_tensor_tensor(
            out=obuf[:, t4, :], in0=tr_n[:, 0:64], scalar=sn, in1=obuf[:, t4, :],
            op0=MUL, op1=ADD,
        )
        if t4 == 3:
            nc.sync.dma_start(
                out=out_p[bh, 512 * lc : 512 * lc + 512, :].rearrange(
                    "(t p) d -> p t d", p=128
                ),
                in_=obufs.pop((bh, lc)),
            )

    # AV emission schedule: normally one item behind the QK/exp stream.
    # The first AV of each L-chunk (js==0, allocating the oac PSUM ring
    # slot) is delayed one extra slot so the previous chunk's PSUM-release
    # copies hide behind two QK pairs instead of one.
    av_sched = [[] for _ in range(len(stream) + 1)]
    for idx, item in enumerate(stream):
        _, lc, js = item
        tgt = idx + 1
        if js == 0 and idx > 0:
            tgt = idx + 2
        av_sched[min(tgt, len(stream))].append(item)

    for idx, item in enumerate(stream):
        emit_qk_exp(item)
        for av_item in av_sched[idx]:
            emit_av(av_item)
        # drain at most one epilogue action every other item: an action
        # adds ~110ns of PE transposes, which would eat the whole
        # PE-vs-ACT slack if popped every item
        if actions and idx % 2 == 0:
            actions.pop(0)()
    for av_item in av_sched[len(stream)]:
        emit_av(av_item)
    while actions:
        actions.pop(0)()


def _host_in_maps(queries, keys, values, keys_null, values_null, tau, delta,
                  soft_mask):
    ident = np.eye(128, dtype=np.float32)[:, 0:65].astype(NPBF16)
    mask = np.triu(np.ones((128, 128))).astype(NPBF16)

    in_maps = []
    for c in range(NCORES):
        b, h0 = c // 4, HPC * (c % 4)
        qt = np.empty((HPC, 128, L), NPBF16)
        kt = np.empty((HPC, 128, S), NPBF16)
        v = np.empty((HPC, 128, NT, D + 1), NPBF16)
        vn = np.empty((HPC, 128, NT, D + 1), NPBF16)
        for bh in range(HPC):
            h = h0 + bh
            qT = queries[b, :, h, :].T.astype(NPBF16)  # [E, L]
            qt[bh, 0:64] = qT
            qt[bh, 64:128] = qT
            kt[bh, 0:64] = keys[b, :, h, :].T.astype(NPBF16)
            kt[bh, 64:128] = keys_null[b, :, h, :].T.astype(NPBF16)
            # [S, D] -> [128 partitions, NT tiles, D(+ones)]
            v[bh, :, :, 0:D] = (
                values[b, :, h, :].reshape(NT, 128, D).transpose(1, 0, 2)
            ).astype(NPBF16)
            v[bh, :, :, D] = 1.0
            vn[bh, :, :, 0:D] = (
                values_null[b, :, h, :].reshape(NT, 128, D).transpose(1, 0, 2)
            ).astype(NPBF16)
            vn[bh, :, :, D] = 1.0
        m_t = np.ascontiguousarray(soft_mask[b].reshape(NT, 128).T)
        csts = np.empty((128, 1 + 3 * NT), np.float32)
        csts[:, 0] = tau[b, 0] / 8.0
        csts[:, 1 : 1 + NT] = (delta[b] / 8.0).reshape(NT, 128).T
        csts[:, 1 + NT : 1 + 2 * NT] = m_t
        csts[:, 1 + 2 * NT :] = 1.0 - m_t
        in_maps.append(
            dict(
                qt=qt, kt=kt, v=v, vn=vn,
                csts=csts,
                ident=ident,
                mask=mask,
            )
        )
    return in_maps


def kernel(queries, keys, values, keys_null, values_null, tau, delta, soft_mask):
    global _NC, LAST_EXEC_NS
    queries = np.asarray(queries, dtype=np.float32)
    keys = np.asarray(keys, dtype=np.float32)
    values = np.asarray(values, dtype=np.float32)
    keys_null = np.asarray(keys_null, dtype=np.float32)
    values_null = np.asarray(values_null, dtype=np.float32)
    tau = np.asarray(tau, dtype=np.float32)
    delta = np.asarray(delta, dtype=np.float32)
    soft_mask = np.asarray(soft_mask, dtype=np.float32)

    if _NC is None:
        _NC = _build()

    in_maps = _host_in_maps(
        queries, keys, values, keys_null, values_null, tau, delta, soft_mask
    )
    res = run_bass_kernel_spmd(
        _NC, in_maps, core_ids=list(range(NCORES)), trace=TRACE
    )
    LAST_EXEC_NS = res.exec_time_ns

    out = np.empty((B, L, H, D), np.float32)
    for c in range(NCORES):
        b, h0 = c // 4, HPC * (c % 4)
        out[b, :, h0 : h0 + HPC, :] = res.results[c]["out"].transpose(1, 0, 2)
    return out
